# revision 9
# baseline (speedup 1.0000x reference)
"""Trainium2 Bass kernel for sparse transposed conv (gather-GEMM-scatter + ReLU).

out[j] = relu(feats[parent(j)] @ weight[koff(j)]), parent(j) = j // 4 exactly,
so feats rows shard contiguously across 8 cores with perfectly disjoint
outputs (no collectives).

Per-core pipeline (core owns 12500 feats rows / 50000 out rows), processed in
4 parent-quarters of 3125 rows so everything stays SBUF-resident:
  1. feats slice pre-transposed on host to [128, rows, 2] (partition p holds
     channels {p, p+128}); DMA one quarter at a time.
  2. Per kernel-offset k: ap_gather (GPSIMD) pulls matched columns into
     [128, m, 2]; weight-stationary fp32 matmuls (lhsT = replicated weight
     half [ci, co], rhs = gathered X [ci, m], N up to 512) accumulate
     psum[co, m]; ReLU-copy into a token-ordered y^T staging buffer
     [128 co, tokens].
  3. Second ap_gather reorders y^T columns into output-row order (each out
     row has exactly one source token); PE transpose flips [co, j] tiles to
     [j, co]; plain sequential HWDGE DMA writes padded regions to HBM
     (no indirect DMA, no descriptor-generation bottleneck, and only one
     GPSIMD ucode library in play).
Host inverts the padded region layout at the end (pure reshape).
"""

import functools
import os

import numpy as np

N_IN = 100_000
K = 8
C_IN = 256
C_OUT = 128
CHILDREN = 4
N_OUT = N_IN * CHILDREN
NCORES = 8
R = N_IN // NCORES        # feats rows per core (12500)
J = N_OUT // NCORES       # out rows per core (50000)
NQ = 4                    # parent quarters per core
RQ = R // NQ              # feats rows per quarter (3125)
JQ = J // NQ              # real out rows per quarter (12500)
JQP = 12544               # padded out rows per quarter (= 7 * 1792)
REG = 1792                # out rows per DMA region (14 tiles of 128)
NREG = JQP // REG         # regions per quarter (7)
JCH = REG                 # out rows per j-gather chunk

LAST_RESULTS = None       # test.py reads exec_time_ns from here


@functools.lru_cache(maxsize=4)
def _build_program(segq: int):
    from contextlib import ExitStack

    import concourse.tile as tile
    from concourse import bacc, mybir
    from concourse.masks import make_identity

    F32 = mybir.dt.float32
    I16 = mybir.dt.int16

    tokq = K * segq
    assert tokq < 32768
    nc = bacc.Bacc("TRN2", target_bir_lowering=False, debug=False,
                   num_devices=NCORES)
    x2_d = nc.dram_tensor("x2", [128, R, 2], F32, kind="ExternalInput").ap()
    w2_d = nc.dram_tensor("w2", [128, K * 2 * C_OUT], F32,
                          kind="ExternalInput").ap()
    gidx_d = nc.dram_tensor("gidx", [128, NQ * tokq // 16], I16,
                            kind="ExternalInput").ap()
    jidx_d = nc.dram_tensor("jidx", [128, NQ * JQP // 16], I16,
                            kind="ExternalInput").ap()
    out_d = nc.dram_tensor("out", [NQ * NREG * 128, REG // 14 * 14],
                           F32, kind="ExternalOutput").ap()

    with tile.TileContext(nc) as tc, ExitStack() as ctx:
        cpool = ctx.enter_context(tc.tile_pool(name="const", bufs=1))
        w2_s = cpool.tile([128, K * 2 * C_OUT], F32)
        gidx_s = cpool.tile([128, NQ * tokq // 16], I16)
        jidx_s = cpool.tile([128, NQ * JQP // 16], I16)
        ident = cpool.tile([128, 128], F32)
        nc.sync.dma_start(out=w2_s[:], in_=w2_d[:])
        nc.sync.dma_start(out=gidx_s[:], in_=gidx_d[:])
        nc.sync.dma_start(out=jidx_s[:], in_=jidx_d[:])
        make_identity(nc, ident[:])

        xpool = ctx.enter_context(tc.tile_pool(name="xq", bufs=2))
        ypool = ctx.enter_context(tc.tile_pool(name="y", bufs=1))
        gpool = ctx.enter_context(tc.tile_pool(name="g", bufs=2))
        jgpool = ctx.enter_context(tc.tile_pool(name="jg", bufs=2))
        ospool = ctx.enter_context(tc.tile_pool(name="os", bufs=3))
        psmm = ctx.enter_context(tc.tile_pool(name="psmm", bufs=4,
                                              space="PSUM"))
        pst = ctx.enter_context(tc.tile_pool(name="pst", bufs=3,
                                             space="PSUM"))

        nrelu = 0
        for q in range(NQ):
            x2q = xpool.tile([128, RQ, 2], F32)
            nc.sync.dma_start(out=x2q[:], in_=x2_d[:, q * RQ:(q + 1) * RQ, :])
            y = ypool.tile([128, tokq], F32)
            for k in range(K):
                # one gather per whole k-segment, then 512-wide matmul chunks
                g = gpool.tile([128, segq, 2], F32)
                base = q * tokq + k * segq
                nc.gpsimd.ap_gather(
                    out_ap=g[:], in_ap=x2q[:],
                    idxs_ap=gidx_s[:, base // 16:(base + segq) // 16],
                    channels=128, num_elems=RQ, d=2, num_idxs=segq)
                done = 0
                while done < segq:
                    cn = min(512, segq - done)
                    ps = psmm.tile([128, 512], F32)
                    nc.tensor.matmul(
                        out=ps[:, :cn],
                        lhsT=w2_s[:, (k * 2 + 0) * C_OUT:(k * 2 + 1) * C_OUT],
                        rhs=g[:, done:done + cn, 0], start=True, stop=False)
                    nc.tensor.matmul(
                        out=ps[:, :cn],
                        lhsT=w2_s[:, (k * 2 + 1) * C_OUT:(k * 2 + 2) * C_OUT],
                        rhs=g[:, done:done + cn, 1], start=False, stop=True)
                    dst = y[:, k * segq + done:k * segq + done + cn]
                    if nrelu % 2 == 0:
                        nc.scalar.activation(
                            out=dst, in_=ps[:, :cn],
                            func=mybir.ActivationFunctionType.Relu)
                    else:
                        nc.vector.tensor_scalar_max(
                            out=dst, in0=ps[:, :cn], scalar1=0.0)
                    nrelu += 1
                    done += cn
            # j-order regather + transpose + linear write-out
            for r in range(NREG):
                yg = jgpool.tile([128, JCH], F32)
                jbase = q * JQP + r * JCH
                nc.gpsimd.ap_gather(
                    out_ap=yg[:], in_ap=y[:],
                    idxs_ap=jidx_s[:, jbase // 16:(jbase + JCH) // 16],
                    channels=128, num_elems=tokq, d=1, num_idxs=JCH)
                ost = ospool.tile([128, JCH], F32)
                # 4 transposes share one psum bank, then one batched copy
                for grp in range(0, JCH // 128, 4):
                    gn = min(4, JCH // 128 - grp)
                    pt = pst.tile([128, 512], F32)
                    for s in range(gn):
                        nc.tensor.transpose(
                            out=pt[:, s * 128:(s + 1) * 128],
                            in_=yg[:, (grp + s) * 128:(grp + s + 1) * 128],
                            identity=ident[:])
                    dst = ost[:, grp * 128:(grp + gn) * 128]
                    if grp % 8 == 0:
                        nc.vector.tensor_copy(out=dst, in_=pt[:, :gn * 128])
                    else:
                        nc.scalar.copy(out=dst, in_=pt[:, :gn * 128])
                row0 = (q * NREG + r) * 128
                nc.sync.dma_start(out=out_d[row0:row0 + 128, :], in_=ost[:])

    nc.compile()
    return nc


def _wrap16(a):
    """token i -> partition i%16, slot i//16; replicated to 128 partitions."""
    return np.tile(a.reshape(len(a) // 16, 16).T, (8, 1))


def _host_prep(feats, weight, gather_idx, scatter_idx, n_out):
    """Build per-core input maps. Pure numpy index munging + layout."""
    feats = np.asarray(feats, dtype=np.float32)
    weight = np.asarray(weight, dtype=np.float32)
    gather_idx = np.asarray(gather_idx, dtype=np.int64)
    scatter_idx = np.asarray(scatter_idx, dtype=np.int64)
    n_out = int(n_out)
    assert feats.shape == (N_IN, C_IN) and weight.shape == (K, C_IN, C_OUT)
    assert n_out == N_OUT

    # real matches per (k, core, quarter), token order = ascending j
    per = {}
    segq = 0
    for k in range(K):
        valid = scatter_idx[k] < n_out
        par = gather_idx[k][valid]
        out_rows = scatter_idx[k][valid]
        assert np.array_equal(par // R, out_rows // J), \
            "match lists are not row-aligned; sharding assumption broken"
        qg = par // RQ          # global quarter id = core*NQ + q
        for c in range(NCORES):
            for q in range(NQ):
                sel = qg == c * NQ + q
                g = par[sel] - (c * NQ + q) * RQ
                j = out_rows[sel] - (c * NQ + q) * JQ
                per[(k, c, q)] = (g, j)
                segq = max(segq, len(g))
    segq = (segq + 127) // 128 * 128
    tokq = K * segq

    feats2 = np.ascontiguousarray(
        feats.reshape(N_IN, 2, 128).transpose(2, 0, 1))
    w2 = np.ascontiguousarray(
        weight.reshape(K, 2, 128, C_OUT).transpose(2, 0, 1, 3)
    ).reshape(128, K * 2 * C_OUT)

    in_maps = []
    for c in range(NCORES):
        gflat = np.zeros(NQ * tokq, dtype=np.int16)
        jflat = np.zeros(NQ * JQP, dtype=np.int16)
        for q in range(NQ):
            tok = np.zeros(JQ, dtype=np.int16)
            covered = np.zeros(JQ, dtype=bool)
            for k in range(K):
                g, j = per[(k, c, q)]
                base = q * tokq + k * segq
                gflat[base:base + len(g)] = g
                tok[j] = (k * segq + np.arange(len(j))).astype(np.int16)
                covered[j] = True
            assert covered.all(), "some output rows have no match"
            jflat[q * JQP:q * JQP + JQ] = tok
        in_maps.append({
            "x2": np.ascontiguousarray(feats2[:, c * R:(c + 1) * R, :]),
            "w2": w2,
            "gidx": _wrap16(gflat),
            "jidx": _wrap16(jflat),
        })
    return in_maps, segq


def _ensure_ntff_hook():
    """This image's antenv lacks axon_hooks; synthesize it so trace=True can
    drive NTFF profiling via the injected libaxon_pjrt.so."""
    import sys
    import types
    try:
        import antenv.axon_hooks  # noqa: F401
        return True
    except ImportError:
        pass
    try:
        import antenv
        from trn_agent_boot.trn_boot import _ntff_profile_via_ctypes
    except ImportError:
        return False
    mod = types.ModuleType("antenv.axon_hooks")
    holder = {}
    mod.set_axon_ntff_profile_hook = lambda h: holder.__setitem__("h", h)
    mod.get_axon_ntff_profile_hook = lambda: holder.get("h")
    sys.modules["antenv.axon_hooks"] = mod
    antenv.axon_hooks = mod
    try:
        h = _ntff_profile_via_ctypes("/opt/axon/libaxon_pjrt.so")
    except OSError:
        h = None
    if h is not None:
        mod.set_axon_ntff_profile_hook(h)
    return True


def kernel(**inputs):
    global LAST_RESULTS
    from concourse.bass_utils import run_bass_kernel_spmd

    in_maps, segq = _host_prep(
        inputs["feats"], inputs["weight"], inputs["gather_idx"],
        inputs["scatter_idx"], inputs["n_out"])
    nc = _build_program(segq)
    trace = bool(int(os.environ.get("KERNEL_TRACE", "0")))
    if trace:
        trace = _ensure_ntff_hook()
    res = run_bass_kernel_spmd(nc, in_maps, list(range(NCORES)), trace=trace)
    LAST_RESULTS = res
    parts = []
    for c in range(NCORES):
        # [NQ*NREG*128, 1792] -> [NQ, NREG, 128, 14, 128] -> j order
        arr = np.asarray(res.results[c]["out"]).reshape(NQ, NREG, 128, 14, 128)
        arr = arr.transpose(0, 1, 3, 2, 4).reshape(NQ, JQP, C_OUT)[:, :JQ]
        parts.append(arr.reshape(J, C_OUT))
    return np.concatenate(parts, axis=0)
